# revision 4
# baseline (speedup 1.0000x reference)
"""Trainium2 Bass kernel for ColorImageLoss (gaussian-blur + bilinear grid
sample + MSE), data-parallel over batch across 8 NeuronCores.

The loss reads the blurred image only at 64 sample points per image.  Each
bilinear sample needs a 2x2 patch of blurred pixels whose 7-tap separable
blur support is an 8x8 patch of the REFLECT-PADDED original image (pad=3).
Host-side reflect padding removes all on-device reflect / border handling:
the window start is simply ws = round(x - 0.5) clamped to [0, 510] (round-
to-nearest-even f32->i32 convert; integer ties land on ws = x0 - 1 with
wx = 1, which yields the identical bilinear result), and the per-axis
8-tap weight vector collapses to

    v[u] = (1 - wx) * kk8[u] + wx * kk8[u-1]   (kk8 = 7-tap kernel + a 0)
         = kk8[u] + wx * (kk8[u-1] - kk8[u])

i.e. two tensor_tensor ops against precomputed meta rows, then one outer
product for the full 8x8 2D weight.

Gather: host builds a banded replica of each padded image - for every
window row start ys in [0, 511) an 8-row band stored [x][c][r] - so a full
8x8x3 patch is ONE contiguous 768B run at ((img*511+ys)*518+xs)*24.  One
indirect-DMA call per sample slot (2 total; HW takes one offset per
partition per call), each moving 128 descriptors of 768B.

Critical path: meta DMA -> 6 DVE ops (scale/clamp, i32 round==floor,
scale-mult, h-add, +base) -> 2 SWDGE gathers -> per-slot apply (mult,
reduce, diff, sq) -> per-slot output DMA (slot0's result ships while
slot1's gather/apply still runs).
"""

import os
import sys

import numpy as np

for _p in ("/opt/trn_rl_repo", "/root/.axon_site/_ro/trn_rl_repo"):
    if os.path.isdir(_p) and _p not in sys.path:
        sys.path.insert(0, _p)

import concourse.bass as bass
import concourse.mybir as mybir
import concourse.tile as tile
from concourse.bass_utils import run_bass_kernel_spmd

# Problem geometry (hardcoded per contract)
B, L, NCH, H, W = 32, 64, 3, 512, 512
NCORES = 8
BPC = B // NCORES            # images per core (4)
NS = BPC * L                 # samples per core (256)
P = 128                      # SBUF partitions
SLOTS = NS // P              # 2 sample slots per partition
KS = 7                       # blur taps
PAD = 3                      # host reflect pad
PADW = W + 2 * PAD           # 518
BANDS = H - 1                # 511 window row starts (ws in [0, 510])
BAND_ROW = PADW * NCH * 8    # 12432 elems per (img, ys)
IMG_BASE = BANDS * BAND_ROW  # 6352752 elems per image
IMG_ELEMS = BPC * IMG_BASE   # banded elems per core
PATCH = 8 * NCH * 8          # 192 elems per gathered patch

f32 = mybir.dt.float32
i32 = mybir.dt.int32
Alu = mybir.AluOpType
Ax = mybir.AxisListType

# meta tensor per-partition layout (f32 columns)
O_POS = 0             # [SLOTS, 2] (x, y)                  -> 4
O_COL = 4             # [SLOTS, 3] color                   -> 6
O_NDK = 10            # [8] kk8[u-1] - kk8[u]              -> 8
O_KK8 = 18            # [8] kk8                            -> 8
O_SCALE = 26          # i32 (24, 12432) x/y offset scales  -> 2
O_BASE = 28           # [SLOTS] i32 image base (bit-cast)  -> 2
META_W = 32


def _gauss_kernel_np():
    x = (np.arange(KS, dtype=np.float32) - (KS - 1) / 2).astype(np.float32)
    k = np.exp(-0.5 * (x / np.float32(1.0)) ** 2).astype(np.float32)
    return (k / k.sum()).astype(np.float32)


def _fap(t, dims, extra_offset=0):
    """AP over tile `t` keeping its partition dim, replacing free dims.

    dims: list of [step, count] in elements; step 0 broadcasts.
    """
    base = t[:] if hasattr(t, "tile") else t
    return bass.AP(
        base.tensor, base.offset + extra_offset,
        [list(base.ap[0])] + [list(d) for d in dims],
    )


def split_multi_waits(nc):
    """Walrus encodes at most ONE sync wait per TPB instruction.  Hoist
    extra waits onto same-engine NoOps inserted directly before the
    instruction (the sequencer executes waits in queue order, so semantics
    are identical)."""
    n_split = 0
    for f in nc.m.functions:
        for blk in f.blocks:
            insts = blk.instructions
            i = 0
            while i < len(insts):
                inst = insts[i]
                si = inst.sync_info
                if si is not None and si.on_wait is not None and len(si.on_wait) > 1:
                    waits = list(si.on_wait)
                    for w in waits[:-1]:
                        nop = mybir.InstNoOp(
                            name=f"{inst.name}-wsplit{n_split}",
                            engine=inst.engine,
                            ins=[],
                            outs=[],
                            sync_info=mybir.SyncInfo(on_wait=[w], on_update=[]),
                        )
                        nc.register_instruction(nop, overwrite=True)
                        insts.insert(i, nop)
                        i += 1
                        n_split += 1
                    inst.sync_info = mybir.SyncInfo(
                        on_wait=[waits[-1]], on_update=list(si.on_update or []))
                i += 1
    return n_split


def build_bass(repeat=1, serialize=True, img_elems=IMG_ELEMS):
    """serialize: chain each repeat body's final result into the next
    body's meta tile so repeats cannot overlap - the repeat-slope then
    measures true per-body latency, not pipelined throughput."""
    nc = bass.Bass("TRN2")

    img = nc.dram_tensor("img", [img_elems, 1], f32, kind="ExternalInput")
    meta = nc.dram_tensor("meta", [P, META_W], f32, kind="ExternalInput")
    parts = [nc.dram_tensor(f"partial{s}", [P, NCH], f32, kind="ExternalOutput")
             for s in range(SLOTS)]

    with tile.TileContext(nc) as tc:
        with tc.tile_pool(name="main", bufs=1) as pool:
            # tiles allocated ONCE; `repeat` bodies reuse them so reps
            # serialize through buffer dependencies (honest latency bench)
            m = pool.tile([P, META_W], f32)
            xh = pool.tile([P, SLOTS, 2], f32)
            wsi = pool.tile([P, SLOTS, 2], i32)
            om = pool.tile([P, SLOTS, 2], i32)
            o1 = pool.tile([P, SLOTS], i32)
            idx = pool.tile([P, SLOTS], i32)
            patches = pool.tile([P, SLOTS, PATCH], f32)
            xyc = pool.tile([P, SLOTS, 2], f32)
            fws = pool.tile([P, SLOTS, 2], f32)
            wxy = pool.tile([P, SLOTS, 2], f32)
            vt = pool.tile([P, SLOTS, 2, 8], f32)
            w2d = pool.tile([P, SLOTS, 64], f32)
            t3 = pool.tile([P, SLOTS, NCH, 64], f32)
            tgt = pool.tile([P, SLOTS, NCH], f32)
            diff = pool.tile([P, SLOTS, NCH], f32)
            sqj = pool.tile([P, SLOTS, NCH], f32)

            pos_ap = _fap(m, [[2, SLOTS], [1, 2]], O_POS)
            scale_ap = _fap(m, [[0, SLOTS], [1, 2]], O_SCALE).bitcast(i32)
            base_ap = _fap(m, [[1, SLOTS]], O_BASE).bitcast(i32)
            ndk_ap = _fap(m, [[0, SLOTS], [0, 2], [1, 8]], O_NDK)
            kk8_ap = _fap(m, [[0, SLOTS], [0, 2], [1, 8]], O_KK8)

            for _rep in range(repeat):
                nc.sync.dma_start(out=m[:], in_=meta[:])

                # ---- index path (gather critical path) ----
                # ws = round(clip(pos*512 - 1, -0.4, 510.5)) == window start;
                # f32->i32 convert rounds-to-nearest, so this IS
                # floor(clip(pos*512-0.5, 0, 511)) up to integer ties, which
                # wx (computed against the same ws) makes harmless.
                nc.vector.tensor_scalar(xh[:], pos_ap, float(W), -1.0,
                                        Alu.mult, Alu.add)
                nc.vector.tensor_scalar(xh[:], xh[:], -0.4, float(W - 2) + 0.5,
                                        Alu.max, Alu.min)
                nc.vector.tensor_copy(wsi[:], xh[:])
                # element offset = ws_x*24 + ws_y*12432 + img_base (i32)
                nc.vector.tensor_tensor(om[:], wsi[:], scale_ap, op=Alu.mult)
                nc.vector.tensor_tensor(
                    o1[:], _fap(om, [[2, SLOTS], [1, 1]]),
                    _fap(om, [[2, SLOTS], [1, 1]], 1), op=Alu.add)
                nc.vector.tensor_tensor(idx[:], o1[:], base_ap, op=Alu.add)

                # ---- gather: one 768B descriptor per partition per slot ----
                for slot in range(SLOTS):
                    nc.gpsimd.indirect_dma_start(
                        out=_fap(patches, [[1, PATCH]], PATCH * slot),
                        out_offset=None,
                        in_=img[:],
                        in_offset=bass.IndirectOffsetOnAxis(
                            ap=_fap(idx, [[1, 1]], slot), axis=0),
                        element_offset=0,
                    )

                # ---- weight path (overlaps gather DMA) ----
                nc.vector.tensor_scalar(xyc[:], pos_ap, float(W), -0.5,
                                        Alu.mult, Alu.add)
                nc.vector.tensor_scalar(xyc[:], xyc[:], 0.0, float(W - 1),
                                        Alu.max, Alu.min)
                nc.vector.tensor_copy(fws[:], wsi[:])
                nc.vector.tensor_tensor(wxy[:], xyc[:], fws[:],
                                        op=Alu.subtract)
                # v[s,ax,u] = kk8[u] + w * (kk8[u-1] - kk8[u])
                wxy_b = _fap(wxy, [[2, SLOTS], [1, 2], [0, 8]])
                nc.vector.tensor_tensor(vt[:], wxy_b, ndk_ap, op=Alu.mult)
                nc.vector.tensor_tensor(vt[:], vt[:], kk8_ap, op=Alu.add)
                # w2d[s, x, r] = v[s, 0, x] * v[s, 1, r]
                vx_b = _fap(vt, [[16, SLOTS], [1, 8], [0, 8]])
                vy_b = _fap(vt, [[16, SLOTS], [0, 8], [1, 8]], 8)
                nc.vector.tensor_tensor(
                    _fap(w2d, [[64, SLOTS], [8, 8], [1, 8]]), vx_b, vy_b,
                    op=Alu.mult)

                # ---- apply weights per slot; patch layout [x][c][r] ----
                for slot in range(SLOTS):
                    pat_ap = _fap(patches, [[8, NCH], [24, 8], [1, 8]],
                                  PATCH * slot)
                    w2d_ap = _fap(w2d, [[0, NCH], [8, 8], [1, 8]], 64 * slot)
                    t3_ap = _fap(t3, [[64, NCH], [8, 8], [1, 8]],
                                 NCH * 64 * slot)
                    nc.vector.tensor_tensor(t3_ap, pat_ap, w2d_ap,
                                            op=Alu.mult)
                    nc.vector.tensor_reduce(
                        out=_fap(tgt, [[1, NCH]], NCH * slot),
                        in_=_fap(t3, [[64, NCH], [1, 64]], NCH * 64 * slot),
                        axis=Ax.X, op=Alu.add)
                    col_ap = _fap(m, [[1, NCH]], O_COL + NCH * slot)
                    diff_ap = _fap(diff, [[1, NCH]], NCH * slot)
                    sq_ap = _fap(sqj, [[1, NCH]], NCH * slot)
                    nc.vector.tensor_tensor(
                        diff_ap, _fap(tgt, [[1, NCH]], NCH * slot), col_ap,
                        op=Alu.subtract)
                    nc.vector.tensor_tensor(sq_ap, diff_ap, diff_ap,
                                            op=Alu.mult)
                    nc.sync.dma_start(out=parts[slot][:], in_=sq_ap)

                if serialize and repeat > 1:
                    # force rep i+1's meta load to wait on rep i's result
                    nc.vector.tensor_copy(
                        _fap(m, [[1, 1]], O_POS),
                        _fap(sqj, [[1, 1]], NCH * (SLOTS - 1)))

    split_multi_waits(nc)
    return nc


def make_meta(pred_shard):
    """Build the per-core [P, META_W] meta tensor from the [BPC, L, 8]
    predictions shard.  Sample i = slot*P + p."""
    flat = np.ascontiguousarray(
        np.asarray(pred_shard).reshape(NS, 8).astype(np.float32))
    meta = np.zeros((P, META_W), dtype=np.float32)
    pos = flat[:, :2].reshape(SLOTS, P, 2).transpose(1, 0, 2)     # [P,SLOTS,2]
    col = flat[:, 5:8].reshape(SLOTS, P, 3).transpose(1, 0, 2)    # [P,SLOTS,3]
    meta[:, O_POS:O_POS + 4] = pos.reshape(P, 4)
    meta[:, O_COL:O_COL + 6] = col.reshape(P, 6)
    kk8 = np.zeros(8, dtype=np.float32)
    kk8[:KS] = _gauss_kernel_np()
    kk8s = np.zeros(8, dtype=np.float32)
    kk8s[1:] = kk8[:7]
    meta[:, O_NDK:O_NDK + 8] = (kk8s - kk8)[None, :]
    meta[:, O_KK8:O_KK8 + 8] = kk8[None, :]
    meta[:, O_SCALE:O_SCALE + 2] = np.array(
        [NCH * 8, BAND_ROW], dtype=np.int32).view(np.float32)[None, :]
    p_idx = np.arange(P)
    base = np.zeros((P, SLOTS), dtype=np.int32)
    for slot in range(SLOTS):
        base[:, slot] = ((slot * P + p_idx) // L).astype(np.int32) * IMG_BASE
    meta[:, O_BASE:O_BASE + SLOTS] = base.view(np.float32)
    return meta


def make_banded(ref_imgs):
    """banded[i, ys, x, c, r] = padded[i, c, ys+r, x] - an 8x8x3 patch at
    (ys, xs) is the contiguous 192-elem run at ((i*511+ys)*518+xs)*24."""
    imgs = np.asarray(ref_imgs, dtype=np.float32)
    padded = np.pad(imgs, ((0, 0), (0, 0), (PAD, PAD), (PAD, PAD)),
                    mode='reflect')
    Yt = np.ascontiguousarray(padded.transpose(0, 2, 3, 1))  # [B, y, x, c]
    banded = np.empty((B, BANDS, PADW, NCH, 8), dtype=np.float32)
    for r in range(8):
        banded[:, :, :, :, r] = Yt[:, r:r + BANDS]
    return banded


def make_in_maps(predictions, ref_imgs):
    banded = make_banded(ref_imgs)
    in_maps = []
    for k in range(NCORES):
        shard = np.ascontiguousarray(
            banded[k * BPC:(k + 1) * BPC]).reshape(-1, 1)
        meta = make_meta(predictions[k * BPC:(k + 1) * BPC])
        in_maps.append({"img": shard, "meta": meta})
    return in_maps


_NC_CACHE = {}


def get_nc():
    if "nc" not in _NC_CACHE:
        _NC_CACHE["nc"] = build_bass()
    return _NC_CACHE["nc"]


def _reduce_results(res):
    total = np.float64(0.0)
    for r in res.results:
        for s in range(SLOTS):
            total += np.float64(r[f"partial{s}"].sum(dtype=np.float64))
    return np.float32(total / (B * L * NCH))


def kernel(predictions, ref_imgs):
    predictions = np.asarray(predictions)
    ref_imgs = np.asarray(ref_imgs)
    nc = get_nc()
    in_maps = make_in_maps(predictions, ref_imgs)
    res = run_bass_kernel_spmd(nc, in_maps, list(range(NCORES)))
    return _reduce_results(res)


def run_profiled(predictions, ref_imgs):
    """Like kernel(), but traces with neuron-profile; returns (loss, results)."""
    predictions = np.asarray(predictions)
    ref_imgs = np.asarray(ref_imgs)
    nc = get_nc()
    in_maps = make_in_maps(predictions, ref_imgs)
    res = run_bass_kernel_spmd(
        nc, in_maps, list(range(NCORES)), trace=True)
    return _reduce_results(res), res
